# revision 27
# baseline (speedup 1.0000x reference)
"""EntNet Trainium2 kernel (8-core data-parallel over batch), v3.

Reference computation (B=64, S=128, L=32, D=100, M=20, V=50000):
  sents = (emb[tokens] * mult).sum(axis=2)            # [B,S,D]
  mem0 = broadcast(keys)                              # [B,M,D]
  per step t: gate = sigmoid(s.mem + s.keys); cand = prelu(mem@Uw.T + keys@Vw.T + s@Ww.T)
              mem = normalize(mem + cand*gate, axis=D)

Kernel strategy per core (8 batches/core), layout [D, rows] with D on
partitions:
  - Embedding gather via gpsimd indirect DMA (one index per partition, 16
    consecutive single-column gathers into one block tile so the SWDGE can
    coalesce descriptor generation), reduced words->sentences with PE
    block-ones matmuls into D-major sents [100, 1024] (t-major cols 8t+b).
  - TWO independent recurrence chains of 4 batches each (rows R2=80),
    interleaved at a half-step offset, so each chain's serial dependency
    cycle is hidden behind the other chain's engine work.
  - Per chain, scale-free form: mem = rho*U, U unnormalized, n = 1/rho:
        l  = rho*(s.U) + s.keys         e = exp(-l)
        U' = (1+e)*U + candf            candf = Uw@U + (Vk + Ws_t)*n
        ss = ||U'||^2 (square+colsum), rho' = exp(-.5 ln ss),
        n' = exp(+.5 ln ss)  (computed in parallel with rho' on ACT)
    Exact renormalization every RESCALE steps resets rho/n to exactly 1.
  - The two gate reductions ride ONE [100,256] f32r colsum matmul per chain
    (U*s | keys*s | zero pad); keys*s and Vk+Ws are materialized once per
    8-step block as strided block ops, not per step. The e/n row-broadcasts
    share one [1,512] f32r matmul per chain per step.
  - f32r (single-pass PE) with moving dim padded to >=256 for the recurrence
    matmuls; fp32 elsewhere.
"""

import os

import numpy as np

B, S, L, D, M, V = 64, 128, 32, 100, 20, 50000
NCORES = 8
BL = B // NCORES            # 8 batches per core
NCH = 2                     # recurrence chains per core
BC = BL // NCH              # 4 batches per chain
R = BL * M                  # 160 rows per core
R2 = BC * M                 # 80 rows per chain
NS = BL * S
NTOK = BL * S * L
NGATH = NTOK // 128         # 256 gather index columns
NBLK = 16                   # sentence blocks (64 sent cols = 8 steps)
RESCALE = 8
RP = 256                    # padded moving dim for f32r matmuls

_prog_cache = {}

_ENGINE_SEM = {"PE": "PE_", "DVE": "DVE_", "Activation": "Activation_",
               "Pool": "Pool_", "SP": "SP_"}


def _strip_redundant_self_waits(nc):
    """Legalize sync waits: walrus rejects >1 sync wait on most instruction
    structs. For any instruction carrying several, hoist all but one onto
    preceding single-wait NoOps on the same engine queue (in-order dispatch
    keeps semantics). The instruction keeps its OWN-engine wait if it has one
    (that wait guards an engine-pipelining RAW hazard and must gate execution,
    not just dispatch).
    """
    import concourse.mybir as mybir
    for fn in nc.m.functions:
        for blk in fn.blocks:
            i = 0
            while i < len(blk.instructions):
                inst = blk.instructions[i]
                si = inst.sync_info() if callable(inst.sync_info) else inst.sync_info
                if (si is not None and si.on_wait and len(si.on_wait) > 1
                        and inst.engine is not None):
                    waits = list(si.on_wait)
                    pref = _ENGINE_SEM.get(inst.engine.name)
                    keep_idx = None
                    for j, w in enumerate(waits):
                        if pref and w.ant_name.startswith(pref):
                            keep_idx = j
                            break
                    kept = [waits.pop(keep_idx)] if keep_idx is not None else []
                    noops = []
                    for w in waits:
                        nop = mybir.InstNoOp(
                            name=nc.get_next_instruction_name(), ins=[], outs=[])
                        nop.engine = inst.engine
                        nop.sync_info = mybir.SyncInfo(on_wait=[w], on_update=[])
                        nc.register_instruction(nop, overwrite=True)
                        noops.append(nop)
                    inst.sync_info = mybir.SyncInfo(
                        on_wait=kept, on_update=list(si.on_update))
                    blk.instructions[i:i] = noops
                    i += len(noops)
                i += 1


def _build_program(a_is_one: bool, mult_is_ones: bool, alpha: float,
                   n_steps: int = S):
    import concourse.bass as bass
    import concourse.tile as tile
    from concourse import mybir
    from contextlib import ExitStack

    f32 = mybir.dt.float32
    f32r = mybir.dt.float32r
    i32 = mybir.dt.int32
    AF = mybir.ActivationFunctionType
    OP = mybir.AluOpType

    nc = bass.Bass(trn_type="TRN2")

    CW = 686 if not mult_is_ones else 586
    tok_d = nc.dram_tensor("tok", [128, NGATH], i32, kind="ExternalInput").ap()
    emb_d = nc.dram_tensor("emb", [V, D], f32, kind="ExternalInput").ap()
    consts_d = nc.dram_tensor("consts", [128, CW], f32r, kind="ExternalInput").ap()
    out_d = nc.dram_tensor("memT", [D, R], f32, kind="ExternalOutput").ap()

    def c(ap):
        # plain-f32 view of an f32r tensor for non-matmul readers
        return ap.bitcast(f32)

    def bcast_mid(ap_2d, n_mid):
        # [P, k] -> [P, n_mid, k] with stride-0 middle dim
        return bass.AP(ap_2d.tensor, ap_2d.offset,
                       [list(ap_2d.ap[0]), [0, n_mid], list(ap_2d.ap[1])])

    def bcast_last(ap_2d, n_last):
        # [P, k] -> [P, k, n_last] with stride-0 last dim
        return bass.AP(ap_2d.tensor, ap_2d.offset,
                       [list(ap_2d.ap[0]), list(ap_2d.ap[1]), [0, n_last]])

    def bcast_mid2(ap_2d, n1, n2):
        # [P, k] -> [P, n1, n2, k] with stride-0 dims 1 and 2
        return bass.AP(ap_2d.tensor, ap_2d.offset,
                       [list(ap_2d.ap[0]), [0, n1], [0, n2], list(ap_2d.ap[1])])

    with tile.TileContext(nc) as tc, ExitStack() as ctx:
        const = ctx.enter_context(tc.tile_pool(name="const", bufs=1))
        gpool = ctx.enter_context(tc.tile_pool(name="gath", bufs=4))
        work = ctx.enter_context(tc.tile_pool(name="work", bufs=2))
        ps_setup = ctx.enter_context(tc.tile_pool(name="ps_setup", bufs=2, space="PSUM"))
        ps_loop = ctx.enter_context(tc.tile_pool(name="ps_loop", bufs=1, space="PSUM"))

        # ---- constants ----
        tok_sb = const.tile([128, NGATH], i32)
        nc.gpsimd.dma_start(out=tok_sb[:], in_=tok_d)
        consts = const.tile([128, CW], f32r)
        nc.sync.dma_start(out=consts[:], in_=consts_d)
        keysT = consts[0:D, 0:M]
        UwT = consts[0:D, 20:120]
        WwT = consts[0:D, 120:220]
        VwT = consts[0:D, 220:320]
        blk = consts[0:128, 320:324]
        onesD = consts[0:D, 324:325]
        ones1 = consts[0:1, 325:425]
        if not mult_is_ones:
            multT = consts[0:128, 586:686]
        zcolD = consts[0:D, CW - 1:CW]
        zcol1 = consts[0:1, CW - 1:CW]

        # ---- Vk = Vw @ keys^T ----
        ps_vk = ps_setup.tile([D, M], f32, tag="pssent", bufs=3, name="ps_vk")
        nc.tensor.matmul(out=ps_vk[:], lhsT=c(VwT[:]), rhs=c(keysT[:]),
                         start=True, stop=True)
        Vk = const.tile([D, M], f32)
        nc.vector.tensor_copy(out=Vk[:], in_=ps_vk[:])

        # ---- gather machinery ----
        sents_b = [const.tile([D, 64], f32, name=f"sents_b{w}")
                   for w in range(NBLK)]
        Ws_b = [const.tile([D, 64], f32, name=f"ws_b{w}") for w in range(NBLK)]
        gtiles = {}
        ps_blks = {}

        def emit_gather(w):
            g = gpool.tile([128, 16, D], f32, tag="g", name=f"g{w}")
            for gi in range(16):
                nc.gpsimd.indirect_dma_start(
                    out=g[:, gi, :],
                    out_offset=None,
                    in_=emb_d,
                    in_offset=bass.IndirectOffsetOnAxis(
                        ap=tok_sb[:, w * 16 + gi:w * 16 + gi + 1], axis=0),
                )
            gtiles[w] = g

        def emit_reduce(w, quarter):
            if quarter == 0:
                ps_blks[w] = ps_setup.tile([D, 64], f32, tag="pssent", bufs=3,
                                           name=f"ps_blk{w}")
            ps_blk = ps_blks[w]
            for gi in range(4 * quarter, 4 * quarter + 4):
                gc = gtiles[w][:, gi, :]
                if not mult_is_ones:
                    gm = gpool.tile([128, D], f32, tag="gm", name=f"gm{w}_{gi}")
                    nc.vector.tensor_tensor(out=gm[:], in0=gc, in1=c(multT[:]),
                                            op=OP.mult)
                    gc = gm[:]
                nc.tensor.matmul(out=ps_blk[:, 4 * gi:4 * gi + 4],
                                 lhsT=gc, rhs=c(blk[:]), start=True, stop=True)

        def emit_finish(w):
            nc.vector.tensor_copy(out=sents_b[w][:], in_=ps_blks[w][:])
            ps_ws = ps_setup.tile([D, 64], f32, tag="pssent", bufs=3,
                                  name=f"ps_ws{w}")
            nc.tensor.matmul(out=ps_ws[:], lhsT=c(WwT[:]), rhs=sents_b[w][:],
                             start=True, stop=True)
            nc.vector.tensor_copy(out=Ws_b[w][:], in_=ps_ws[:])

        # ---- fixed per-chain state tiles ----
        U_t = [[const.tile([D, RP], f32r, name=f"U{ci}_{i}") for i in range(3)]
               for ci in range(NCH)]
        # ksB step-slice layout [100, 256]: [0:80]=U*s (per step),
        # [80:160]=keys*s (per block), [160:256]=zero pad
        ksB = [[const.tile([D, 8, RP], f32r, name=f"ksB{ci}_{i}")
                for i in range(2)] for ci in range(NCH)]
        vwB = [[const.tile([D, 8, R2], f32, name=f"vwB{ci}_{i}")
                for i in range(2)] for ci in range(NCH)]
        # en layout [1, 512]: [0:80]=n, [256:336]=e, rest zero (bcen rhs)
        en_t = [[const.tile([1, 2 * RP], f32r, name=f"en{ci}_{i}")
                 for i in range(2)] for ci in range(NCH)]
        rho_t = [[const.tile([1, RP], f32r, name=f"rho{ci}_{i}")
                  for i in range(2)] for ci in range(NCH)]
        sq_t = [[const.tile([D, RP], f32r, name=f"sq{ci}_{i}")
                 for i in range(2)] for ci in range(NCH)]
        for ci in range(NCH):
            for i in range(3):
                nc.vector.tensor_copy(out=U_t[ci][i][:, R2:RP],
                                      in_=bcast_last(zcolD, RP - R2))
            for i in range(2):
                nc.vector.tensor_copy(out=sq_t[ci][i][:, R2:RP],
                                      in_=bcast_last(zcolD, RP - R2))
            for i in range(2):
                nc.vector.tensor_copy(
                    out=ksB[ci][i][:, :, 2 * R2:RP],
                    in_=bass.AP(zcolD.tensor, zcolD.offset,
                                [list(zcolD.ap[0]), [0, 8], [0, RP - 2 * R2]]))
                nc.vector.tensor_copy(out=en_t[ci][i][:, 0:RP],
                                      in_=bcast_last(zcol1, RP))
                nc.vector.tensor_copy(out=en_t[ci][i][:, RP + R2:2 * RP],
                                      in_=bcast_last(zcol1, RP - R2))
                nc.vector.tensor_copy(out=rho_t[ci][i][:, R2:RP],
                                      in_=bcast_last(zcol1, RP - R2))
            nc.vector.tensor_copy(
                out=U_t[ci][0][:, 0:R2].rearrange("d (b m) -> d b m", m=M),
                in_=bcast_mid(c(keysT), BC))

        VE, GE, AC = nc.vector, nc.gpsimd, nc.scalar

        def emit_ksvw_block(ci, w, on_pool):
            """keys*s and Vk+Ws for all 8 steps of block w, chain ci."""
            E = GE if on_pool else VE
            kd = ksB[ci][w % 2][:, :, R2:2 * R2].rearrange(
                "p a (b m) -> p a b m", m=M)
            sb = sents_b[w]
            s_ap = bass.AP(sb.tensor, sb[:, 4 * ci:].offset,
                           [list(sb.ap[0]), [8, 8], [1, BC], [0, M]])
            E.tensor_tensor(out=kd, in0=bcast_mid2(c(keysT), 8, BC),
                            in1=s_ap, op=OP.mult)
            vd = vwB[ci][w % 2][:].rearrange("p a (b m) -> p a b m", m=M)
            wb = Ws_b[w]
            ws_ap = bass.AP(wb.tensor, wb[:, 4 * ci:].offset,
                            [list(wb.ap[0]), [8, 8], [1, BC], [0, M]])
            E.tensor_tensor(out=vd, in0=bcast_mid2(Vk[:], 8, BC),
                            in1=ws_ap, op=OP.add)

        # per-chain python state
        ucur = [0] * NCH
        unext_idx = [0] * NCH
        ures_idx = [0] * NCH
        rho = [None] * NCH
        has_n = [False] * NCH
        psA_ps = [None] * NCH
        bcen_ps = [None] * NCH

        def prep(ci, t):
            """psA matmul + mgt product (need only prev-step state)."""
            w, c8 = t // 8, t % 8
            Ucur = U_t[ci][ucur[ci]]
            psA_ps[ci] = ps_loop.tile([D, RP], f32, tag="psA", bufs=2,
                                      name=f"psA{ci}_{t}")
            nc.tensor.matmul(out=psA_ps[ci][:], lhsT=UwT[:], rhs=Ucur[:],
                             start=True, stop=True)
            sb = sents_b[w]
            s_sl = bass.AP(sb.tensor, sb[:, 8 * c8 + 4 * ci:].offset,
                           [list(sb.ap[0]), [1, BC], [0, M]])
            VE.tensor_tensor(
                out=ksB[ci][w % 2][:, c8, 0:R2].rearrange(
                    "d (b m) -> d b m", m=M),
                in0=c(Ucur[:, 0:R2]).rearrange("d (b m) -> d b m", m=M),
                in1=s_sl, op=OP.mult)

        def gate(ci, t):
            """psmg matmul, l, e."""
            w, c8, k = t // 8, t % 8, t % 2
            psmg = ps_loop.tile([1, RP], f32, tag="rowps", bufs=2,
                                name=f"psmg{ci}_{t}")
            nc.tensor.matmul(out=psmg[:], lhsT=onesD[:],
                             rhs=ksB[ci][w % 2][:, c8, :], start=True, stop=True)
            l1 = work.tile([1, R2], f32, tag=f"l1_{ci}", name=f"l1{ci}_{t}")
            if rho[ci] is not None:
                VE.tensor_tensor(out=l1[:], in0=psmg[:, 0:R2], in1=rho[ci],
                                 op=OP.mult)
            else:
                VE.tensor_copy(out=l1[:], in_=psmg[:, 0:R2])
            l_sb = work.tile([1, R2], f32, tag=f"l_{ci}", name=f"l{ci}_{t}")
            VE.tensor_tensor(out=l_sb[:], in0=psmg[:, R2:2 * R2], in1=l1[:],
                             op=OP.add)
            AC.activation(out=en_t[ci][k][:, RP:RP + R2], in_=l_sb[:],
                          func=AF.Exp, scale=-1.0)

        def back_a(ci, t):
            """update: bcen, candf, V, U2; advances the U rotation."""
            w, c8, k = t // 8, t % 8, t % 2
            Ucur = U_t[ci][ucur[ci]]
            unext_idx[ci] = (ucur[ci] + 1) % 3
            Unext = U_t[ci][unext_idx[ci]]
            bcen_ps[ci] = ps_loop.tile([D, 336], f32, tag="bcen",
                                       name=f"bcen{ci}_{t}")
            nc.tensor.matmul(out=bcen_ps[ci][:], lhsT=ones1[:],
                             rhs=en_t[ci][k][:, 0:336], start=True, stop=True)
            vw_sl = vwB[ci][w % 2][:, c8, :]
            candf = work.tile([D, R2], f32, tag=f"candf_{ci}",
                              name=f"candf{ci}_{t}")
            if has_n[ci]:
                c1 = work.tile([D, R2], f32, tag=f"c1_{ci}", name=f"c1{ci}_{t}")
                VE.tensor_tensor(out=c1[:], in0=vw_sl,
                                 in1=bcen_ps[ci][:, 0:R2], op=OP.mult)
                VE.tensor_tensor(out=candf[:], in0=psA_ps[ci][:, 0:R2],
                                 in1=c1[:], op=OP.add)
            else:
                VE.tensor_tensor(out=candf[:], in0=psA_ps[ci][:, 0:R2],
                                 in1=vw_sl, op=OP.add)
            cand = candf
            if not a_is_one:
                candp = work.tile([D, R2], f32, tag=f"candp_{ci}",
                                  name=f"candp{ci}_{t}")
                AC.activation(out=candp[:], in_=candf[:], func=AF.Prelu,
                              alpha=float(alpha))
                cand = candp
            V_sb = work.tile([D, R2], f32, tag=f"V_{ci}", name=f"V{ci}_{t}")
            VE.scalar_tensor_tensor(out=V_sb[:], in0=bcen_ps[ci][:, RP:RP + R2],
                                    scalar=1.0, in1=c(Ucur[:, 0:R2]),
                                    op0=OP.add, op1=OP.mult)
            VE.tensor_tensor(out=Unext[:, 0:R2], in0=V_sb[:], in1=cand[:],
                             op=OP.add)
            if (t + 1) % RESCALE == 0:
                ures_idx[ci] = (ucur[ci] + 2) % 3
                ucur[ci] = ures_idx[ci]
            else:
                ucur[ci] = unext_idx[ci]

        def sqop(ci, t, on_pool):
            k = t % 2
            Unext = U_t[ci][unext_idx[ci]]
            SE = GE if on_pool else VE
            SE.tensor_tensor(out=sq_t[ci][k][:, 0:R2], in0=c(Unext[:, 0:R2]),
                             in1=c(Unext[:, 0:R2]), op=OP.mult)

        def normr(ci, t):
            """colsum -> ln -> {rho, n}; rescale multiply."""
            k = t % 2
            Unext = U_t[ci][unext_idx[ci]]
            psss = ps_loop.tile([1, RP], f32, tag="rowps", bufs=2,
                                name=f"psss{ci}_{t}")
            nc.tensor.matmul(out=psss[:], lhsT=onesD[:], rhs=sq_t[ci][k][:],
                             start=True, stop=True)
            lnss = work.tile([1, R2], f32, tag=f"lnss_{ci}", name=f"lnss{ci}_{t}")
            AC.activation(out=lnss[:], in_=psss[:, 0:R2], func=AF.Ln)
            kn = (t + 1) % 2
            AC.activation(out=rho_t[ci][kn][:, 0:R2], in_=lnss[:],
                          func=AF.Exp, scale=-0.5)
            if (t + 1) % RESCALE == 0:
                psbcr = ps_loop.tile([D, RP], f32, tag="bcen",
                                     name=f"psbcr{ci}_{t}")
                nc.tensor.matmul(out=psbcr[:], lhsT=ones1[:],
                                 rhs=rho_t[ci][kn][:], start=True, stop=True)
                Un2 = U_t[ci][ures_idx[ci]]
                VE.tensor_tensor(out=Un2[:, 0:R2], in0=psbcr[:, 0:R2],
                                 in1=c(Unext[:, 0:R2]), op=OP.mult)
                rho[ci] = None
                has_n[ci] = False
            else:
                AC.activation(out=en_t[ci][kn][:, 0:R2], in_=lnss[:],
                              func=AF.Exp, scale=0.5)
                rho[ci] = c(rho_t[ci][kn][:, 0:R2])
                has_n[ci] = True

        # ---- prologue: blocks 0 and 1, ks/vw for block 0 ----
        for w in (0, 1):
            emit_gather(w)
            for q in range(4):
                emit_reduce(w, q)
            emit_finish(w)
        for ci in range(NCH):
            emit_ksvw_block(ci, 0, on_pool=False)

        # ---- main loop: chains interleaved at half-step offset ----
        for t in range(n_steps):
            w, c8 = t // 8, t % 8
            pool_free = w + 2 >= NBLK
            if c8 == 2 and w + 2 < NBLK:
                emit_gather(w + 2)
            if c8 >= 4 and w + 2 < NBLK:
                emit_reduce(w + 2, c8 - 4)
                if c8 == 7:
                    emit_finish(w + 2)
            if c8 == 5 and w + 1 < NBLK:
                emit_ksvw_block(0, w + 1, on_pool=pool_free)
            if c8 == 6 and w + 1 < NBLK:
                emit_ksvw_block(1, w + 1, on_pool=pool_free)

            prep(0, t)
            if t % RESCALE != 0 or t == 0:
                prep(1, t)
            gate(0, t)
            if t > 0:
                normr(1, t - 1)
            if t % RESCALE == 0 and t > 0:
                prep(1, t)
            back_a(0, t)
            gate(1, t)
            sqop(0, t, pool_free)
            normr(0, t)
            back_a(1, t)
            sqop(1, t, pool_free)
        normr(1, n_steps - 1)

        # ---- output ----
        for ci in range(NCH):
            Ufin = U_t[ci][ucur[ci]]
            if n_steps % RESCALE == 0:
                nc.sync.dma_start(out=out_d[:, R2 * ci:R2 * (ci + 1)],
                                  in_=c(Ufin[:, 0:R2]))
            else:
                psbcr = ps_loop.tile([D, RP], f32, tag="bcen",
                                     name=f"psbcrf{ci}")
                rfin = rho_t[ci][n_steps % 2]
                nc.tensor.matmul(out=psbcr[:], lhsT=ones1[:], rhs=rfin[:],
                                 start=True, stop=True)
                memT = work.tile([D, R2], f32, tag=f"memT_{ci}",
                                 name=f"memT{ci}")
                nc.vector.tensor_tensor(out=memT[:], in0=psbcr[:, 0:R2],
                                        in1=c(Ufin[:, 0:R2]), op=OP.mult)
                nc.sync.dma_start(out=out_d[:, R2 * ci:R2 * (ci + 1)],
                                  in_=memT[:])

    _strip_redundant_self_waits(nc)
    return nc


def _stage_inputs(tokens, emb, keys, mult, Uw, Vw, Ww, prelu_a):
    """Host-side sharding/layout prep. Returns (in_maps, flags)."""
    tokens = np.asarray(tokens)
    emb = np.ascontiguousarray(np.asarray(emb, dtype=np.float32))
    keys = np.asarray(keys, dtype=np.float32)
    mult = np.asarray(mult, dtype=np.float32)
    a = float(np.asarray(prelu_a).reshape(-1)[0])
    a_is_one = (a == 1.0)
    mult_is_ones = bool(np.all(mult == 1.0))

    CW = 686 if not mult_is_ones else 586
    consts = np.zeros((128, CW), np.float32)
    consts[0:D, 0:M] = keys.T
    consts[0:D, 20:120] = np.asarray(Uw, np.float32).T        # lhsT for Uw@mem
    consts[0:D, 120:220] = np.asarray(Ww, np.float32).T
    consts[0:D, 220:320] = np.asarray(Vw, np.float32).T
    consts[0:128, 320:324] = np.kron(np.eye(4, dtype=np.float32),
                                     np.ones((32, 1), np.float32))
    consts[0:D, 324:325] = 1.0                                # onesD
    consts[0:1, 325:425] = 1.0                                # ones1
    if not mult_is_ones:
        consts[0:128, 586:686] = np.tile(mult, (4, 1))

    in_maps = []
    for cr in range(NCORES):
        tc_ = tokens[cr * BL:(cr + 1) * BL]                   # [8, S, L]
        # sentence-major rows with t-major sentence order: row j = 8t+b
        tokflat = np.ascontiguousarray(tc_.transpose(1, 0, 2)).reshape(NS, L)
        # tok_staged[p, col] = token of sentence 4*col + p//32, word p%32
        tok_staged = np.ascontiguousarray(
            tokflat.reshape(2 * S, 4, L).transpose(1, 2, 0)).reshape(128, 2 * S)
        in_maps.append({"tok": np.ascontiguousarray(tok_staged, np.int32),
                        "emb": emb, "consts": consts})
    return in_maps, a_is_one, mult_is_ones, a


def kernel(tokens, emb, keys, mult, Uw, Vw, Ww, prelu_a, _trace=False):
    from concourse.bass_utils import run_bass_kernel_spmd

    in_maps, a_is_one, mult_is_ones, a = _stage_inputs(
        tokens, emb, keys, mult, Uw, Vw, Ww, prelu_a)

    key = (a_is_one, mult_is_ones, a)
    if key not in _prog_cache:
        _prog_cache[key] = _build_program(a_is_one, mult_is_ones, a)
    nc = _prog_cache[key]

    res = run_bass_kernel_spmd(nc, in_maps, list(range(NCORES)), trace=_trace)
    out = np.empty((B, M, D), dtype=np.float32)
    for cr in range(NCORES):
        memT = res.results[cr]["memT"]                         # [D, R]
        out[cr * BL:(cr + 1) * BL] = memT.reshape(D, BL, M).transpose(1, 2, 0)
    kernel._last_results = res
    return out


# revision 28
# speedup vs baseline: 1.0307x; 1.0307x over previous
"""EntNet Trainium2 kernel (8-core data-parallel over batch), v3.

Reference computation (B=64, S=128, L=32, D=100, M=20, V=50000):
  sents = (emb[tokens] * mult).sum(axis=2)            # [B,S,D]
  mem0 = broadcast(keys)                              # [B,M,D]
  per step t: gate = sigmoid(s.mem + s.keys); cand = prelu(mem@Uw.T + keys@Vw.T + s@Ww.T)
              mem = normalize(mem + cand*gate, axis=D)

Kernel strategy per core (8 batches/core), layout [D, rows] with D on
partitions:
  - Embedding gather via gpsimd indirect DMA (one index per partition, 16
    consecutive single-column gathers into one block tile so the SWDGE can
    coalesce descriptor generation), reduced words->sentences with PE
    block-ones matmuls into D-major sents [100, 1024] (t-major cols 8t+b).
  - TWO independent recurrence chains of 4 batches each (rows R2=80),
    interleaved at a half-step offset, so each chain's serial dependency
    cycle is hidden behind the other chain's engine work.
  - Per chain, scale-free form: mem = rho*U, U unnormalized, n = 1/rho:
        l  = rho*(s.U) + s.keys         e = exp(-l)
        U' = (1+e)*U + candf            candf = Uw@U + (Vk + Ws_t)*n
        ss = ||U'||^2 (square+colsum), rho' = exp(-.5 ln ss),
        n' = exp(+.5 ln ss)  (computed in parallel with rho' on ACT)
    Exact renormalization every RESCALE steps resets rho/n to exactly 1.
  - The two gate reductions ride ONE [100,256] f32r colsum matmul per chain
    (U*s | keys*s | zero pad); keys*s and Vk+Ws are materialized once per
    8-step block as strided block ops, not per step. The e/n row-broadcasts
    share one [1,512] f32r matmul per chain per step.
  - f32r (single-pass PE) with moving dim padded to >=256 for the recurrence
    matmuls; fp32 elsewhere.
"""

import os

import numpy as np

B, S, L, D, M, V = 64, 128, 32, 100, 20, 50000
NCORES = 8
BL = B // NCORES            # 8 batches per core
NCH = 2                     # recurrence chains per core
BC = BL // NCH              # 4 batches per chain
R = BL * M                  # 160 rows per core
R2 = BC * M                 # 80 rows per chain
NS = BL * S
NTOK = BL * S * L
NGATH = NTOK // 128         # 256 gather index columns
NBLK = 16                   # sentence blocks (64 sent cols = 8 steps)
RESCALE = 8
RP = 256                    # padded moving dim for f32r matmuls

_prog_cache = {}

_ENGINE_SEM = {"PE": "PE_", "DVE": "DVE_", "Activation": "Activation_",
               "Pool": "Pool_", "SP": "SP_"}


def _strip_redundant_self_waits(nc):
    """Legalize sync waits: walrus rejects >1 sync wait on most instruction
    structs. For any instruction carrying several, hoist all but one onto
    preceding single-wait NoOps on the same engine queue (in-order dispatch
    keeps semantics). The instruction keeps its OWN-engine wait if it has one
    (that wait guards an engine-pipelining RAW hazard and must gate execution,
    not just dispatch).
    """
    import concourse.mybir as mybir
    for fn in nc.m.functions:
        for blk in fn.blocks:
            i = 0
            while i < len(blk.instructions):
                inst = blk.instructions[i]
                si = inst.sync_info() if callable(inst.sync_info) else inst.sync_info
                if (si is not None and si.on_wait and len(si.on_wait) > 1
                        and inst.engine is not None):
                    waits = list(si.on_wait)
                    pref = _ENGINE_SEM.get(inst.engine.name)
                    keep_idx = None
                    for j, w in enumerate(waits):
                        if pref and w.ant_name.startswith(pref):
                            keep_idx = j
                            break
                    kept = [waits.pop(keep_idx)] if keep_idx is not None else []
                    noops = []
                    for w in waits:
                        nop = mybir.InstNoOp(
                            name=nc.get_next_instruction_name(), ins=[], outs=[])
                        nop.engine = inst.engine
                        nop.sync_info = mybir.SyncInfo(on_wait=[w], on_update=[])
                        nc.register_instruction(nop, overwrite=True)
                        noops.append(nop)
                    inst.sync_info = mybir.SyncInfo(
                        on_wait=kept, on_update=list(si.on_update))
                    blk.instructions[i:i] = noops
                    i += len(noops)
                i += 1


def _build_program(a_is_one: bool, mult_is_ones: bool, alpha: float,
                   n_steps: int = S):
    import concourse.bass as bass
    import concourse.tile as tile
    from concourse import mybir
    from contextlib import ExitStack

    f32 = mybir.dt.float32
    f32r = mybir.dt.float32r
    i32 = mybir.dt.int32
    AF = mybir.ActivationFunctionType
    OP = mybir.AluOpType

    nc = bass.Bass(trn_type="TRN2")

    CW = 686 if not mult_is_ones else 586
    tok_d = nc.dram_tensor("tok", [128, NGATH], i32, kind="ExternalInput").ap()
    emb_d = nc.dram_tensor("emb", [V, D], f32, kind="ExternalInput").ap()
    consts_d = nc.dram_tensor("consts", [128, CW], f32r, kind="ExternalInput").ap()
    out_d = nc.dram_tensor("memT", [D, R], f32, kind="ExternalOutput").ap()

    def c(ap):
        # plain-f32 view of an f32r tensor for non-matmul readers
        return ap.bitcast(f32)

    def bcast_mid(ap_2d, n_mid):
        # [P, k] -> [P, n_mid, k] with stride-0 middle dim
        return bass.AP(ap_2d.tensor, ap_2d.offset,
                       [list(ap_2d.ap[0]), [0, n_mid], list(ap_2d.ap[1])])

    def bcast_last(ap_2d, n_last):
        # [P, k] -> [P, k, n_last] with stride-0 last dim
        return bass.AP(ap_2d.tensor, ap_2d.offset,
                       [list(ap_2d.ap[0]), list(ap_2d.ap[1]), [0, n_last]])

    def bcast_mid2(ap_2d, n1, n2):
        # [P, k] -> [P, n1, n2, k] with stride-0 dims 1 and 2
        return bass.AP(ap_2d.tensor, ap_2d.offset,
                       [list(ap_2d.ap[0]), [0, n1], [0, n2], list(ap_2d.ap[1])])

    with tile.TileContext(nc) as tc, ExitStack() as ctx:
        const = ctx.enter_context(tc.tile_pool(name="const", bufs=1))
        gpool = ctx.enter_context(tc.tile_pool(name="gath", bufs=4))
        work = ctx.enter_context(tc.tile_pool(name="work", bufs=2))
        ps_setup = ctx.enter_context(tc.tile_pool(name="ps_setup", bufs=2, space="PSUM"))
        ps_loop = ctx.enter_context(tc.tile_pool(name="ps_loop", bufs=1, space="PSUM"))

        # ---- constants ----
        tok_sb = const.tile([128, NGATH], i32)
        nc.gpsimd.dma_start(out=tok_sb[:], in_=tok_d)
        consts = const.tile([128, CW], f32r)
        nc.sync.dma_start(out=consts[:], in_=consts_d)
        keysT = consts[0:D, 0:M]
        UwT = consts[0:D, 20:120]
        WwT = consts[0:D, 120:220]
        VwT = consts[0:D, 220:320]
        blk = consts[0:128, 320:324]
        onesD = consts[0:D, 324:325]
        ones1 = consts[0:1, 325:425]
        if not mult_is_ones:
            multT = consts[0:128, 586:686]
        zcolD = consts[0:D, CW - 1:CW]
        zcol1 = consts[0:1, CW - 1:CW]

        # ---- Vk = Vw @ keys^T ----
        ps_vk = ps_setup.tile([D, M], f32, tag="pssent", bufs=3, name="ps_vk")
        nc.tensor.matmul(out=ps_vk[:], lhsT=c(VwT[:]), rhs=c(keysT[:]),
                         start=True, stop=True)
        Vk = const.tile([D, M], f32)
        nc.vector.tensor_copy(out=Vk[:], in_=ps_vk[:])

        # ---- gather machinery ----
        sents_b = [const.tile([D, 64], f32, name=f"sents_b{w}")
                   for w in range(NBLK)]
        Ws_b = [const.tile([D, 64], f32, name=f"ws_b{w}") for w in range(NBLK)]
        gtiles = {}
        ps_blks = {}

        def emit_gather(w):
            g = gpool.tile([128, 16, D], f32, tag="g", name=f"g{w}")
            for gi in range(16):
                nc.gpsimd.indirect_dma_start(
                    out=g[:, gi, :],
                    out_offset=None,
                    in_=emb_d,
                    in_offset=bass.IndirectOffsetOnAxis(
                        ap=tok_sb[:, w * 16 + gi:w * 16 + gi + 1], axis=0),
                )
            gtiles[w] = g

        def emit_reduce(w, quarter):
            if quarter == 0:
                ps_blks[w] = ps_setup.tile([D, 64], f32, tag="pssent", bufs=3,
                                           name=f"ps_blk{w}")
            ps_blk = ps_blks[w]
            for gi in range(4 * quarter, 4 * quarter + 4):
                gc = gtiles[w][:, gi, :]
                if not mult_is_ones:
                    gm = gpool.tile([128, D], f32, tag="gm", name=f"gm{w}_{gi}")
                    nc.vector.tensor_tensor(out=gm[:], in0=gc, in1=c(multT[:]),
                                            op=OP.mult)
                    gc = gm[:]
                nc.tensor.matmul(out=ps_blk[:, 4 * gi:4 * gi + 4],
                                 lhsT=gc, rhs=c(blk[:]), start=True, stop=True)

        def emit_finish(w):
            nc.vector.tensor_copy(out=sents_b[w][:], in_=ps_blks[w][:])
            ps_ws = ps_setup.tile([D, 64], f32, tag="pssent", bufs=3,
                                  name=f"ps_ws{w}")
            nc.tensor.matmul(out=ps_ws[:], lhsT=c(WwT[:]), rhs=sents_b[w][:],
                             start=True, stop=True)
            nc.vector.tensor_copy(out=Ws_b[w][:], in_=ps_ws[:])

        # ---- fixed per-chain state tiles ----
        U_t = [[const.tile([D, RP], f32r, name=f"U{ci}_{i}") for i in range(3)]
               for ci in range(NCH)]
        # ksB step-slice layout [100, 256]: [0:80]=U*s (per step),
        # [80:160]=keys*s (per block), [160:256]=zero pad
        ksB = [[const.tile([D, 8, RP], f32r, name=f"ksB{ci}_{i}")
                for i in range(2)] for ci in range(NCH)]
        vwB = [[const.tile([D, 8, R2], f32, name=f"vwB{ci}_{i}")
                for i in range(2)] for ci in range(NCH)]
        # en layout [1, 512]: [0:80]=n, [256:336]=e, rest zero (bcen rhs)
        en_t = [[const.tile([1, 2 * RP], f32r, name=f"en{ci}_{i}")
                 for i in range(2)] for ci in range(NCH)]
        rho_t = [[const.tile([1, RP], f32r, name=f"rho{ci}_{i}")
                  for i in range(2)] for ci in range(NCH)]
        sq_t = [[const.tile([D, RP], f32r, name=f"sq{ci}_{i}")
                 for i in range(2)] for ci in range(NCH)]
        for ci in range(NCH):
            for i in range(3):
                nc.vector.tensor_copy(out=U_t[ci][i][:, R2:RP],
                                      in_=bcast_last(zcolD, RP - R2))
            for i in range(2):
                nc.vector.tensor_copy(out=sq_t[ci][i][:, R2:RP],
                                      in_=bcast_last(zcolD, RP - R2))
            for i in range(2):
                nc.vector.tensor_copy(
                    out=ksB[ci][i][:, :, 2 * R2:RP],
                    in_=bass.AP(zcolD.tensor, zcolD.offset,
                                [list(zcolD.ap[0]), [0, 8], [0, RP - 2 * R2]]))
                nc.vector.tensor_copy(out=en_t[ci][i][:, 0:RP],
                                      in_=bcast_last(zcol1, RP))
                nc.vector.tensor_copy(out=en_t[ci][i][:, RP + R2:2 * RP],
                                      in_=bcast_last(zcol1, RP - R2))
                nc.vector.tensor_copy(out=rho_t[ci][i][:, R2:RP],
                                      in_=bcast_last(zcol1, RP - R2))
            nc.vector.tensor_copy(
                out=U_t[ci][0][:, 0:R2].rearrange("d (b m) -> d b m", m=M),
                in_=bcast_mid(c(keysT), BC))

        VE, GE, AC = nc.vector, nc.gpsimd, nc.scalar

        def emit_ksvw_block(ci, w, on_pool):
            """keys*s and Vk+Ws for all 8 steps of block w, chain ci."""
            E = GE if on_pool else VE
            kd = ksB[ci][w % 2][:, :, R2:2 * R2].rearrange(
                "p a (b m) -> p a b m", m=M)
            sb = sents_b[w]
            s_ap = bass.AP(sb.tensor, sb[:, 4 * ci:].offset,
                           [list(sb.ap[0]), [8, 8], [1, BC], [0, M]])
            E.tensor_tensor(out=kd, in0=bcast_mid2(c(keysT), 8, BC),
                            in1=s_ap, op=OP.mult)
            vd = vwB[ci][w % 2][:].rearrange("p a (b m) -> p a b m", m=M)
            wb = Ws_b[w]
            ws_ap = bass.AP(wb.tensor, wb[:, 4 * ci:].offset,
                            [list(wb.ap[0]), [8, 8], [1, BC], [0, M]])
            E.tensor_tensor(out=vd, in0=bcast_mid2(Vk[:], 8, BC),
                            in1=ws_ap, op=OP.add)

        # per-chain python state
        ucur = [0] * NCH
        unext_idx = [0] * NCH
        ures_idx = [0] * NCH
        rho = [None] * NCH
        has_n = [False] * NCH
        psA_ps = [None] * NCH
        bcen_ps = [None] * NCH

        def prep(ci, t):
            """psA matmul + mgt product (need only prev-step state)."""
            w, c8 = t // 8, t % 8
            Ucur = U_t[ci][ucur[ci]]
            psA_ps[ci] = ps_loop.tile([D, RP], f32, tag="psA", bufs=2,
                                      name=f"psA{ci}_{t}")
            nc.tensor.matmul(out=psA_ps[ci][:], lhsT=UwT[:], rhs=Ucur[:],
                             start=True, stop=True)
            sb = sents_b[w]
            s_sl = bass.AP(sb.tensor, sb[:, 8 * c8 + 4 * ci:].offset,
                           [list(sb.ap[0]), [1, BC], [0, M]])
            VE.tensor_tensor(
                out=ksB[ci][w % 2][:, c8, 0:R2].rearrange(
                    "d (b m) -> d b m", m=M),
                in0=c(Ucur[:, 0:R2]).rearrange("d (b m) -> d b m", m=M),
                in1=s_sl, op=OP.mult)

        def gate(ci, t):
            """psmg matmul, l, e."""
            w, c8, k = t // 8, t % 8, t % 2
            psmg = ps_loop.tile([1, RP], f32, tag="rowps", bufs=2,
                                name=f"psmg{ci}_{t}")
            nc.tensor.matmul(out=psmg[:], lhsT=onesD[:],
                             rhs=ksB[ci][w % 2][:, c8, :], start=True, stop=True)
            l1 = work.tile([1, R2], f32, tag=f"l1_{ci}", name=f"l1{ci}_{t}")
            if rho[ci] is not None:
                VE.tensor_tensor(out=l1[:], in0=psmg[:, 0:R2], in1=rho[ci],
                                 op=OP.mult)
            else:
                VE.tensor_copy(out=l1[:], in_=psmg[:, 0:R2])
            l_sb = work.tile([1, R2], f32, tag=f"l_{ci}", name=f"l{ci}_{t}")
            VE.tensor_tensor(out=l_sb[:], in0=psmg[:, R2:2 * R2], in1=l1[:],
                             op=OP.add)
            AC.activation(out=en_t[ci][k][:, RP:RP + R2], in_=l_sb[:],
                          func=AF.Exp, scale=-1.0)

        def back_a(ci, t):
            """update: bcen, candf, V, U2; advances the U rotation."""
            w, c8, k = t // 8, t % 8, t % 2
            Ucur = U_t[ci][ucur[ci]]
            unext_idx[ci] = (ucur[ci] + 1) % 3
            Unext = U_t[ci][unext_idx[ci]]
            bcen_ps[ci] = ps_loop.tile([D, 336], f32, tag="bcen",
                                       name=f"bcen{ci}_{t}")
            nc.tensor.matmul(out=bcen_ps[ci][:], lhsT=ones1[:],
                             rhs=en_t[ci][k][:, 0:336], start=True, stop=True)
            vw_sl = vwB[ci][w % 2][:, c8, :]
            candf = work.tile([D, R2], f32, tag=f"candf_{ci}",
                              name=f"candf{ci}_{t}")
            if has_n[ci]:
                c1 = work.tile([D, R2], f32, tag=f"c1_{ci}", name=f"c1{ci}_{t}")
                VE.tensor_tensor(out=c1[:], in0=vw_sl,
                                 in1=bcen_ps[ci][:, 0:R2], op=OP.mult)
                VE.tensor_tensor(out=candf[:], in0=psA_ps[ci][:, 0:R2],
                                 in1=c1[:], op=OP.add)
            else:
                VE.tensor_tensor(out=candf[:], in0=psA_ps[ci][:, 0:R2],
                                 in1=vw_sl, op=OP.add)
            cand = candf
            if not a_is_one:
                candp = work.tile([D, R2], f32, tag=f"candp_{ci}",
                                  name=f"candp{ci}_{t}")
                AC.activation(out=candp[:], in_=candf[:], func=AF.Prelu,
                              alpha=float(alpha))
                cand = candp
            V_sb = work.tile([D, R2], f32, tag=f"V_{ci}", name=f"V{ci}_{t}")
            VE.scalar_tensor_tensor(out=V_sb[:], in0=bcen_ps[ci][:, RP:RP + R2],
                                    scalar=1.0, in1=c(Ucur[:, 0:R2]),
                                    op0=OP.add, op1=OP.mult)
            VE.tensor_tensor(out=Unext[:, 0:R2], in0=V_sb[:], in1=cand[:],
                             op=OP.add)
            if (t + 1) % RESCALE == 0:
                ures_idx[ci] = (ucur[ci] + 2) % 3
                ucur[ci] = ures_idx[ci]
            else:
                ucur[ci] = unext_idx[ci]

        def sqop(ci, t, on_pool):
            k = t % 2
            Unext = U_t[ci][unext_idx[ci]]
            SE = GE if on_pool else VE
            SE.tensor_tensor(out=sq_t[ci][k][:, 0:R2], in0=c(Unext[:, 0:R2]),
                             in1=c(Unext[:, 0:R2]), op=OP.mult)

        def normr(ci, t):
            """colsum -> ln -> {rho, n}; rescale multiply."""
            k = t % 2
            Unext = U_t[ci][unext_idx[ci]]
            psss = ps_loop.tile([1, RP], f32, tag="rowps", bufs=2,
                                name=f"psss{ci}_{t}")
            nc.tensor.matmul(out=psss[:], lhsT=onesD[:], rhs=sq_t[ci][k][:],
                             start=True, stop=True)
            lnss = work.tile([1, R2], f32, tag=f"lnss_{ci}", name=f"lnss{ci}_{t}")
            AC.activation(out=lnss[:], in_=psss[:, 0:R2], func=AF.Ln)
            kn = (t + 1) % 2
            AC.activation(out=rho_t[ci][kn][:, 0:R2], in_=lnss[:],
                          func=AF.Exp, scale=-0.5)
            if (t + 1) % RESCALE == 0:
                psbcr = ps_loop.tile([D, RP], f32, tag="psA", bufs=2,
                                     name=f"psbcr{ci}_{t}")
                nc.tensor.matmul(out=psbcr[:], lhsT=ones1[:],
                                 rhs=rho_t[ci][kn][:], start=True, stop=True)
                Un2 = U_t[ci][ures_idx[ci]]
                VE.tensor_tensor(out=Un2[:, 0:R2], in0=psbcr[:, 0:R2],
                                 in1=c(Unext[:, 0:R2]), op=OP.mult)
                rho[ci] = None
                has_n[ci] = False
            else:
                AC.activation(out=en_t[ci][kn][:, 0:R2], in_=lnss[:],
                              func=AF.Exp, scale=0.5)
                rho[ci] = c(rho_t[ci][kn][:, 0:R2])
                has_n[ci] = True

        # ---- prologue: block 0 fully; block 1 gather only ----
        emit_gather(0)
        emit_gather(1)
        for q in range(4):
            emit_reduce(0, q)
        emit_finish(0)
        for ci in range(NCH):
            emit_ksvw_block(ci, 0, on_pool=False)

        # ---- main loop: chains interleaved at half-step offset ----
        for t in range(n_steps):
            w, c8 = t // 8, t % 8
            pool_free = w + 2 >= NBLK
            if w == 0 and c8 < 4:
                emit_reduce(1, c8)
                if c8 == 3:
                    emit_finish(1)
            if c8 == 2 and w + 2 < NBLK:
                emit_gather(w + 2)
            if c8 >= 4 and w + 2 < NBLK:
                emit_reduce(w + 2, c8 - 4)
                if c8 == 7:
                    emit_finish(w + 2)
            if c8 == 5 and w + 1 < NBLK:
                emit_ksvw_block(0, w + 1, on_pool=pool_free)
            if c8 == 6 and w + 1 < NBLK:
                emit_ksvw_block(1, w + 1, on_pool=pool_free)

            prep(0, t)
            if t % RESCALE != 0 or t == 0:
                prep(1, t)
            gate(0, t)
            if t > 0:
                normr(1, t - 1)
            if t % RESCALE == 0 and t > 0:
                prep(1, t)
            back_a(0, t)
            gate(1, t)
            sqop(0, t, pool_free)
            normr(0, t)
            back_a(1, t)
            sqop(1, t, pool_free)
        normr(1, n_steps - 1)

        # ---- output ----
        for ci in range(NCH):
            Ufin = U_t[ci][ucur[ci]]
            if n_steps % RESCALE == 0:
                nc.sync.dma_start(out=out_d[:, R2 * ci:R2 * (ci + 1)],
                                  in_=c(Ufin[:, 0:R2]))
            else:
                psbcr = ps_loop.tile([D, RP], f32, tag="bcen",
                                     name=f"psbcrf{ci}")
                rfin = rho_t[ci][n_steps % 2]
                nc.tensor.matmul(out=psbcr[:], lhsT=ones1[:], rhs=rfin[:],
                                 start=True, stop=True)
                memT = work.tile([D, R2], f32, tag=f"memT_{ci}",
                                 name=f"memT{ci}")
                nc.vector.tensor_tensor(out=memT[:], in0=psbcr[:, 0:R2],
                                        in1=c(Ufin[:, 0:R2]), op=OP.mult)
                nc.sync.dma_start(out=out_d[:, R2 * ci:R2 * (ci + 1)],
                                  in_=memT[:])

    _strip_redundant_self_waits(nc)
    return nc


def _stage_inputs(tokens, emb, keys, mult, Uw, Vw, Ww, prelu_a):
    """Host-side sharding/layout prep. Returns (in_maps, flags)."""
    tokens = np.asarray(tokens)
    emb = np.ascontiguousarray(np.asarray(emb, dtype=np.float32))
    keys = np.asarray(keys, dtype=np.float32)
    mult = np.asarray(mult, dtype=np.float32)
    a = float(np.asarray(prelu_a).reshape(-1)[0])
    a_is_one = (a == 1.0)
    mult_is_ones = bool(np.all(mult == 1.0))

    CW = 686 if not mult_is_ones else 586
    consts = np.zeros((128, CW), np.float32)
    consts[0:D, 0:M] = keys.T
    consts[0:D, 20:120] = np.asarray(Uw, np.float32).T        # lhsT for Uw@mem
    consts[0:D, 120:220] = np.asarray(Ww, np.float32).T
    consts[0:D, 220:320] = np.asarray(Vw, np.float32).T
    consts[0:128, 320:324] = np.kron(np.eye(4, dtype=np.float32),
                                     np.ones((32, 1), np.float32))
    consts[0:D, 324:325] = 1.0                                # onesD
    consts[0:1, 325:425] = 1.0                                # ones1
    if not mult_is_ones:
        consts[0:128, 586:686] = np.tile(mult, (4, 1))

    in_maps = []
    for cr in range(NCORES):
        tc_ = tokens[cr * BL:(cr + 1) * BL]                   # [8, S, L]
        # sentence-major rows with t-major sentence order: row j = 8t+b
        tokflat = np.ascontiguousarray(tc_.transpose(1, 0, 2)).reshape(NS, L)
        # tok_staged[p, col] = token of sentence 4*col + p//32, word p%32
        tok_staged = np.ascontiguousarray(
            tokflat.reshape(2 * S, 4, L).transpose(1, 2, 0)).reshape(128, 2 * S)
        in_maps.append({"tok": np.ascontiguousarray(tok_staged, np.int32),
                        "emb": emb, "consts": consts})
    return in_maps, a_is_one, mult_is_ones, a


def kernel(tokens, emb, keys, mult, Uw, Vw, Ww, prelu_a, _trace=False):
    from concourse.bass_utils import run_bass_kernel_spmd

    in_maps, a_is_one, mult_is_ones, a = _stage_inputs(
        tokens, emb, keys, mult, Uw, Vw, Ww, prelu_a)

    key = (a_is_one, mult_is_ones, a)
    if key not in _prog_cache:
        _prog_cache[key] = _build_program(a_is_one, mult_is_ones, a)
    nc = _prog_cache[key]

    res = run_bass_kernel_spmd(nc, in_maps, list(range(NCORES)), trace=_trace)
    out = np.empty((B, M, D), dtype=np.float32)
    for cr in range(NCORES):
        memT = res.results[cr]["memT"]                         # [D, R]
        out[cr * BL:(cr + 1) * BL] = memT.reshape(D, BL, M).transpose(1, 2, 0)
    kernel._last_results = res
    return out
